# revision 1
# baseline (speedup 1.0000x reference)
# GNN edge-apply MLP kernel for Trainium2 (Bass/Tile), 8-core SPMD.
#
# reference semantics:
#   feat = concat(node_feats[src], node_feats[dst], axis=1)      # [E, 2048]
#   h    = relu(feat @ W1 + b1)                                  # [E, 1024]
#   out  = h @ W2 + b2                                           # [E, 1024]
#
# Sharding: edges are split evenly across 8 cores (8192 each); the node
# feature table and MLP weights are replicated.  Inside each core, edges are
# processed in 64 tiles of 128.  Edge e of the core shard maps to
# (p, t) = (e // 64, e % 64): tile t holds edges {p*64 + t}, so the index
# tile loads and the output stores are contiguous per partition.
#
# Per tile:
#   - indirect-DMA gather of 128 src rows and 128 dst rows ([128, 1024] f32)
#   - PE transposes (16 x [128,128]) -> featT chunks (K on partitions)
#   - 32 accumulating matmuls (N=512) -> psum1 [128e, 1024h]
#   - DVE +b1, ACT relu -> h
#   - 8 PE transposes -> hT
#   - 16 matmuls -> psum2 [128e, 1024]
#   - DVE +b2 -> out tile -> HWDGE store
import os
import sys

import numpy as np

for _p in ("/opt/trn_rl_repo",):
    if _p not in sys.path:
        sys.path.insert(0, _p)

N_NODES = 50000
D_NODE = 1024
D_HID = 1024
N_CORES = 8
E_TOTAL = 65536
E_CORE = E_TOTAL // N_CORES
P = 128

# compute dtype for matmuls/transposes: "f32r" (full-rate fp32 storage),
# "f32" (exact, quarter-rate), selected via env for experiments.
COMPUTE = os.environ.get("KERNEL_COMPUTE", "f32r")


def build_nc(e_core=E_CORE, n_nodes=N_NODES, compute=COMPUTE):
    import concourse.bass as bass
    import concourse.mybir as mybir
    import concourse.tile as tile
    from concourse import bacc
    from concourse.masks import make_identity

    f32 = mybir.dt.float32
    i32 = mybir.dt.int32
    # All matmul/transpose operand tiles are allocated in store_dt; the
    # producing ops (DMA cast, DVE copies, ACT relu) perform the rounding
    # the FP32r verifier requires.
    store_dt = {"f32": f32, "f32r": mybir.dt.float32r, "bf16": mybir.dt.bfloat16}[
        compute
    ]

    def mm_view(ap):
        return ap

    T = e_core // P  # edge tiles per core
    KD = (2 * D_NODE) // P  # 16 contraction chunks, layer 1
    KH = D_HID // P  # 8 contraction chunks, layer 2
    NH = 512  # matmul moving free dim (one PSUM bank of fp32)

    nc = bacc.Bacc(None, target_bir_lowering=False)

    # In f32r mode the fp32 payloads are declared float32r in DRAM (identical
    # bits, np dtype float32) so SBUF loads are cast-free and can use HWDGE.
    tbl_dt = store_dt if compute == "f32r" else f32
    nf = nc.dram_tensor("node_feats", [n_nodes, D_NODE], tbl_dt, kind="ExternalInput")
    w1 = nc.dram_tensor("W1", [2 * D_NODE, D_HID], tbl_dt, kind="ExternalInput")
    w2 = nc.dram_tensor("W2", [D_HID, D_HID], tbl_dt, kind="ExternalInput")
    b1 = nc.dram_tensor("b1", [D_HID], f32, kind="ExternalInput")
    b2 = nc.dram_tensor("b2", [D_HID], f32, kind="ExternalInput")
    src = nc.dram_tensor("src", [e_core], i32, kind="ExternalInput")
    dst = nc.dram_tensor("dst", [e_core], i32, kind="ExternalInput")
    out = nc.dram_tensor("out", [e_core, D_HID], f32, kind="ExternalOutput")

    nf_ap = nf.ap()
    out_v = out.ap().rearrange("(p t) h -> p t h", t=T)

    with tile.TileContext(nc) as tc:
        with (
            tc.tile_pool(name="const", bufs=1) as const_pool,
            tc.tile_pool(name="wpool", bufs=1) as wpool,
            tc.tile_pool(name="gather", bufs=4) as gather_pool,
            tc.tile_pool(name="work", bufs=2) as work_pool,
            tc.tile_pool(name="featT", bufs=8) as featT_pool,
            tc.tile_pool(name="hT", bufs=4) as hT_pool,
            tc.tile_pool(name="outp", bufs=3) as out_pool,
            tc.tile_pool(name="pstf", bufs=2, space="PSUM") as psTf_pool,
            tc.tile_pool(name="psth", bufs=1, space="PSUM") as psTh_pool,
            tc.tile_pool(name="psmm", bufs=4, space="PSUM") as psmm_pool,
        ):
            # ---- constants / weights ----
            if store_dt == f32:
                ident = const_pool.tile([P, P], f32)
                make_identity(nc, ident[:])
            else:
                # memset/affine_select don't accept f32r/bf16 set values;
                # build in f32 and round via a DVE copy.
                ident_f32 = const_pool.tile([P, P], f32)
                make_identity(nc, ident_f32[:])
                ident = const_pool.tile([P, P], store_dt)
                nc.vector.tensor_copy(ident[:], ident_f32[:])

            idx_src = const_pool.tile([P, T], i32)
            idx_dst = const_pool.tile([P, T], i32)
            nc.sync.dma_start(idx_src[:], src.ap().rearrange("(p t) -> p t", t=T))
            nc.sync.dma_start(idx_dst[:], dst.ap().rearrange("(p t) -> p t", t=T))

            # W1 as [128, KD, 1024]: chunk k rows k*128..k*128+127 of W1.
            # Loaded per-chunk so the first layer-1 matmuls aren't gated on
            # the full 12.6MB weight transfer.
            # Interleave W1/W2 chunk loads 2:1 so layer-2 weights arrive
            # before tile 0 reaches layer 2 instead of after all of W1.
            w_eng = nc.sync if compute == "f32r" else nc.gpsimd
            w1_sb = wpool.tile([P, KD, D_HID], store_dt)
            w1_v = w1.ap().rearrange("(k p) h -> p k h", p=P)
            w2_sb = wpool.tile([P, KH, D_HID], store_dt)
            w2_v = w2.ap().rearrange("(k p) h -> p k h", p=P)
            for k in range(KH):
                w_eng.dma_start(w1_sb[:, 2 * k], w1_v[:, 2 * k])
                w_eng.dma_start(w1_sb[:, 2 * k + 1], w1_v[:, 2 * k + 1])
                w_eng.dma_start(w2_sb[:, k], w2_v[:, k])

            # biases broadcast to all partitions
            b1_bc = const_pool.tile([P, D_HID], f32)
            nc.sync.dma_start(b1_bc[:], b1.ap()[None, :].to_broadcast([P, D_HID]))
            b2_bc = const_pool.tile([P, D_HID], f32)
            nc.sync.dma_start(b2_bc[:], b2.ap()[None, :].to_broadcast([P, D_HID]))

            # Two-deep software pipeline: the PE stream per iteration is
            # [C1(t) h-transposes | A(t+2) gather-transposes + L1 | C2(t) L2]
            # so every PE->DVE->PE handoff (featT/hT copies, relu) has a full
            # stage of independent PE work to hide under, which also keeps
            # the HAM clock gate warm.  PSUM: psTf 2 + psTh 2 + psmm 4 = 8.
            def stage_A(t):
                """Gathers, feature transposes, layer-1 matmuls -> psum1 halves."""
                src_f = gather_pool.tile([P, D_NODE], store_dt, tag="srcf")
                nc.gpsimd.indirect_dma_start(
                    out=src_f[:],
                    out_offset=None,
                    in_=nf_ap[:],
                    in_offset=bass.IndirectOffsetOnAxis(
                        ap=idx_src[:, t : t + 1], axis=0
                    ),
                )
                dst_f = gather_pool.tile([P, D_NODE], store_dt, tag="dstf")
                nc.gpsimd.indirect_dma_start(
                    out=dst_f[:],
                    out_offset=None,
                    in_=nf_ap[:],
                    in_offset=bass.IndirectOffsetOnAxis(
                        ap=idx_dst[:, t : t + 1], axis=0
                    ),
                )

                featT = []
                for g in range(4):
                    psT = psTf_pool.tile([P, 4 * P], store_dt, tag="psT")
                    for j in range(4):
                        k = 4 * g + j
                        blk = (
                            src_f[:, k * P : (k + 1) * P]
                            if k < 8
                            else dst_f[:, (k - 8) * P : (k - 7) * P]
                        )
                        nc.tensor.transpose(
                            mm_view(psT[:, j * P : (j + 1) * P]),
                            mm_view(blk),
                            mm_view(ident[:]),
                        )
                    fT = featT_pool.tile([P, 4 * P], store_dt, tag="featT")
                    nc.vector.tensor_copy(fT[:], psT[:])
                    featT.append(fT)

                halves = []
                for half in range(D_HID // NH):
                    ps1h = psmm_pool.tile([P, NH], f32, tag="psmm")
                    for k in range(KD):
                        fT = featT[k // 4][:, (k % 4) * P : (k % 4 + 1) * P]
                        nc.tensor.matmul(
                            ps1h[:],
                            mm_view(fT),
                            mm_view(w1_sb[:, k, half * NH : (half + 1) * NH]),
                            start=(k == 0),
                            stop=(k == KD - 1),
                        )
                    halves.append(ps1h)
                return halves

            def stage_B(t, halves):
                """psum1 + b1 -> relu -> h_relu (SBUF)."""
                h_relu = work_pool.tile([P, D_HID], store_dt, tag="hrelu")
                for half, ps1h in enumerate(halves):
                    h_add = work_pool.tile([P, NH], f32, tag="hadd")
                    nc.vector.tensor_add(
                        h_add[:], ps1h[:], b1_bc[:, half * NH : (half + 1) * NH]
                    )
                    nc.scalar.activation(
                        h_relu[:, half * NH : (half + 1) * NH],
                        h_add[:],
                        mybir.ActivationFunctionType.Relu,
                    )
                return h_relu

            def stage_C1(t, h_relu):
                """h transposes into one 2-bank PSUM tile, one DVE copy out."""
                psT = psTh_pool.tile([P, KH * P], store_dt, tag="psTh")
                for k in range(KH):
                    nc.tensor.transpose(
                        mm_view(psT[:, k * P : (k + 1) * P]),
                        mm_view(h_relu[:, k * P : (k + 1) * P]),
                        mm_view(ident[:]),
                    )
                hT = hT_pool.tile([P, KH * P], store_dt, tag="hT")
                nc.vector.tensor_copy(hT[:], psT[:])
                return hT

            def stage_C2(t, hT):
                """Layer-2 matmuls, +b2, store."""
                halves = []
                for half in range(D_HID // NH):
                    ps2h = psmm_pool.tile([P, NH], f32, tag="psmm")
                    for k in range(KH):
                        nc.tensor.matmul(
                            ps2h[:],
                            mm_view(hT[:, k * P : (k + 1) * P]),
                            mm_view(w2_sb[:, k, half * NH : (half + 1) * NH]),
                            start=(k == 0),
                            stop=(k == KH - 1),
                        )
                    halves.append(ps2h)

                o_sb = out_pool.tile([P, D_HID], f32, tag="osb")
                for half, ps2h in enumerate(halves):
                    nc.vector.tensor_add(
                        o_sb[:, half * NH : (half + 1) * NH],
                        ps2h[:],
                        b2_bc[:, half * NH : (half + 1) * NH],
                    )
                nc.sync.dma_start(out_v[:, t, :], o_sb[:])

            # pipeline: A(0); A(1); B(0); then per t: C1(t) B(t+1) A(t+2) C2(t)
            ps1_halves = {0: stage_A(0)}
            if T > 1:
                ps1_halves[1] = stage_A(1)
            h_relus = {0: stage_B(0, ps1_halves.pop(0))}
            for t in range(T):
                hT = stage_C1(t, h_relus.pop(t))
                if t + 1 < T:
                    h_relus[t + 1] = stage_B(t + 1, ps1_halves.pop(t + 1))
                if t + 2 < T:
                    ps1_halves[t + 2] = stage_A(t + 2)
                stage_C2(t, hT)

    nc.compile()
    return nc


LAST_RESULTS = None


def kernel(**inputs):
    global LAST_RESULTS
    from concourse.bass_utils import run_bass_kernel_spmd

    node_feats = np.ascontiguousarray(np.asarray(inputs["node_feats"], np.float32))
    W1 = np.ascontiguousarray(np.asarray(inputs["W1"], np.float32))
    W2 = np.ascontiguousarray(np.asarray(inputs["W2"], np.float32))
    b1 = np.ascontiguousarray(np.asarray(inputs["b1"], np.float32))
    b2 = np.ascontiguousarray(np.asarray(inputs["b2"], np.float32))
    src = np.ascontiguousarray(np.asarray(inputs["src"]).astype(np.int32))
    dst = np.ascontiguousarray(np.asarray(inputs["dst"]).astype(np.int32))

    nc = build_nc()

    in_maps = []
    for c in range(N_CORES):
        sl = slice(c * E_CORE, (c + 1) * E_CORE)
        in_maps.append(
            {
                "node_feats": node_feats,
                "W1": W1,
                "W2": W2,
                "b1": b1,
                "b2": b2,
                "src": src[sl],
                "dst": dst[sl],
            }
        )

    trace = bool(int(os.environ.get("KERNEL_TRACE", "0")))
    kw = {}
    if trace and bool(int(os.environ.get("KERNEL_TRACE_ALL", "0"))):
        kw["trace_cores"] = list(range(N_CORES))
    res = run_bass_kernel_spmd(
        nc, in_maps, core_ids=list(range(N_CORES)), trace=trace, **kw
    )
    LAST_RESULTS = res
    return np.concatenate([r["out"] for r in res.results], axis=0)



# revision 9
# speedup vs baseline: 1.0083x; 1.0083x over previous
# GNN edge-apply MLP kernel for Trainium2 (Bass/Tile), 8-core SPMD.
#
# reference semantics:
#   feat = concat(node_feats[src], node_feats[dst], axis=1)      # [E, 2048]
#   h    = relu(feat @ W1 + b1)                                  # [E, 1024]
#   out  = h @ W2 + b2                                           # [E, 1024]
#
# Sharding: edges split evenly across 8 cores (8192 each); node table and
# weights replicated.
#
# v5 design.  The f32r baseline was PE-bound at 93%: every f32r matmul
# self-loads its stationary operand (f32r cannot use standalone LDWEIGHTS,
# serializing a ~150ns weight load with each stream) and 24 PE transposes
# ran per 128-edge tile.  Here the PE runs ONLY the 48 unavoidable N=512
# bf16 matmul streams per 128-edge tile (~224ns each sustained; standalone
# LDWEIGHTS + FWL + the PE's pull-ahead hide the weight loads):
#
#   - edges are processed in supertiles of 512 (4 tiles of 128).
#   - gather: 4+4 indirect DMAs of 128 rows each, f32 (exact).
#   - cast f32->bf16 on the scalar/vector engines.
#   - bounce: plain DMA of the bf16 rows to a DRAM scratch tile, then a
#     DMA-XBAR transpose DRAM->SBUF (the hardware-supported direction;
#     SBUF->SBUF XBAR transposes return corrupt data on HW) produces
#     featT [128, 8, 512]: feature dim on partitions, edges on free.
#   - layer 1 computed TRANSPOSED: psum[hid_m, 512e] += W1_blk[k,m]^T @
#     featT_k (stationary = W1 block, moving = featT, N=512).  Layer 1
#     output is then already h^T, so NO h transpose exists; bias+relu is
#     fused into one scalar-engine op per chunk (bias is per-partition in
#     this layout), writing hT [128, 8, 512] bf16 straight to SBUF.
#   - layer 2: stationary = hT edge-slices, moving = W2 (N=512), psum
#     [128e, 1024] -> +b2 on DVE -> store via an AP that undoes the
#     supertile edge interleave.
#   - prefetch chains are issued 2 supertiles (~80us of PE work) ahead;
#     layer 2 trails layer 1 by one supertile.
import os
import sys

import numpy as np

for _p in ("/opt/trn_rl_repo",):
    if _p not in sys.path:
        sys.path.insert(0, _p)

N_NODES = 50000
D_NODE = 1024
D_HID = 1024
N_CORES = 8
E_TOTAL = 65536
E_CORE = E_TOTAL // N_CORES
P = 128
SUP = 4  # tiles of 128 edges per supertile


def build_nc(e_core=E_CORE, n_nodes=N_NODES):
    import concourse.bass as bass
    import concourse.mybir as mybir
    import concourse.tile as tile
    from concourse import bacc

    f32 = mybir.dt.float32
    bf16 = mybir.dt.bfloat16
    i32 = mybir.dt.int32

    T = e_core // P  # 64 edge tiles per core
    S = T // SUP  # 16 supertiles per core
    ES = SUP * P  # 512 edges per supertile
    KD = (2 * D_NODE) // P  # 16 contraction chunks, layer 1
    KH = D_HID // P  # 8 contraction chunks, layer 2
    NH = 512  # matmul moving free dim (one PSUM bank of fp32)

    nc = bacc.Bacc(None, target_bir_lowering=False)

    nf = nc.dram_tensor("node_feats", [n_nodes, D_NODE], f32, kind="ExternalInput")
    w1 = nc.dram_tensor("W1", [2 * D_NODE, D_HID], f32, kind="ExternalInput")
    w2 = nc.dram_tensor("W2", [D_HID, D_HID], f32, kind="ExternalInput")
    b1 = nc.dram_tensor("b1", [D_HID], f32, kind="ExternalInput")
    b2 = nc.dram_tensor("b2", [D_HID], f32, kind="ExternalInput")
    src = nc.dram_tensor("src", [e_core], i32, kind="ExternalInput")
    dst = nc.dram_tensor("dst", [e_core], i32, kind="ExternalInput")
    out = nc.dram_tensor("out", [e_core, D_HID], f32, kind="ExternalOutput")

    nf_ap = nf.ap()
    # edge e = p*T + s*SUP + j -> supertile s, scratch row r = p*SUP + j.
    # The store for edge-group g of supertile s covers rows {p in
    # [32g, 32g+32), j in [0, 4)}; iterating (p, j, h) row-major matches the
    # SBUF tile's partition order local = 4*(p - 32g) + j.
    out_r = out.ap().rearrange("(p s j) h -> s p j h", s=S, j=SUP)

    with tile.TileContext(nc) as tc:
        with (
            tc.tile_pool(name="const", bufs=1) as const_pool,
            tc.tile_pool(name="wpool", bufs=1) as wpool,
            tc.tile_pool(name="gather", bufs=2) as gather_pool,
            tc.tile_pool(name="gbf", bufs=2) as gbf_pool,
            tc.tile_pool(name="featT", bufs=2) as featT_pool,
            tc.tile_pool(name="hT", bufs=2) as hT_pool,
            tc.tile_pool(name="outp", bufs=2) as out_pool,
            tc.tile_pool(name="scr", bufs=2, space="DRAM") as scr_pool,
            tc.tile_pool(name="psT", bufs=2, space="PSUM") as psT_pool,
            tc.tile_pool(name="ps2", bufs=2, space="PSUM") as ps2_pool,
        ):
            idx_src = const_pool.tile([P, T], i32)
            idx_dst = const_pool.tile([P, T], i32)
            nc.sync.dma_start(idx_src[:], src.ap().rearrange("(p t) -> p t", t=T))
            nc.sync.dma_start(idx_dst[:], dst.ap().rearrange("(p t) -> p t", t=T))

            # W1 as [128, KD, 1024] bf16: chunk k holds rows k*128..k*128+127.
            # Casting loads go through the software DGE (gpsimd).  Interleave
            # W1/W2 chunk loads 2:1 so layer-2 weights arrive early.
            w1_sb = wpool.tile([P, KD, D_HID], bf16)
            w1_v = w1.ap().rearrange("(k p) h -> p k h", p=P)
            w2_sb = wpool.tile([P, KH, D_HID], bf16)
            w2_v = w2.ap().rearrange("(k p) h -> p k h", p=P)
            for k in range(KH):
                nc.gpsimd.dma_start(w1_sb[:, 2 * k], w1_v[:, 2 * k])
                nc.gpsimd.dma_start(w1_sb[:, 2 * k + 1], w1_v[:, 2 * k + 1])
                nc.gpsimd.dma_start(w2_sb[:, k], w2_v[:, k])

            # b1 per-partition: b1_pp[p, m] = b1[m*128 + p]
            b1_pp = const_pool.tile([P, KH], f32)
            nc.sync.dma_start(b1_pp[:], b1.ap().rearrange("(m p) -> p m", p=P))
            # b2 broadcast to all partitions
            b2_bc = const_pool.tile([P, D_HID], f32)
            nc.sync.dma_start(b2_bc[:], b2.ap()[None, :].to_broadcast([P, D_HID]))

            def stage_P(s):
                """Prefetch: gathers -> casts -> DRAM bounce -> XBAR featT."""
                fTs = []
                for name, idx, ceng in (
                    ("s", idx_src, nc.scalar),
                    ("d", idx_dst, nc.vector),
                ):
                    g_f = gather_pool.tile([P, SUP, D_NODE], f32, tag="g" + name)
                    for j in range(SUP):
                        nc.gpsimd.indirect_dma_start(
                            out=g_f[:, j],
                            out_offset=None,
                            in_=nf_ap[:],
                            in_offset=bass.IndirectOffsetOnAxis(
                                ap=idx[:, SUP * s + j : SUP * s + j + 1], axis=0
                            ),
                        )
                    g_bf = gbf_pool.tile([P, SUP, D_NODE], bf16, tag="b" + name)
                    if ceng is nc.scalar:
                        nc.scalar.activation(
                            g_bf[:], g_f[:], mybir.ActivationFunctionType.Copy
                        )
                    else:
                        nc.vector.tensor_copy(g_bf[:], g_f[:])
                    scr = scr_pool.tile([ES, D_NODE], bf16, tag="scr" + name)
                    nc.sync.dma_start(
                        scr[:].rearrange("(p j) h -> p j h", j=SUP), g_bf[:]
                    )
                    fT = featT_pool.tile([P, KH, ES], bf16, tag="fT" + name)
                    nc.sync.dma_start(fT[:], scr[:], transpose=True)
                    fTs.append(fT)
                return fTs

            def stage_L1(s, fTs):
                """Layer 1, transposed: psum[hid_m, 512e]; fused bias+relu."""
                fT_s, fT_d = fTs
                hT = hT_pool.tile([P, KH, ES], bf16, tag="hT")
                for pair in range(KH // 2):
                    ps = psT_pool.tile([P, 2 * NH], f32, tag="psT")
                    for half in range(2):
                        m = 2 * pair + half
                        for k in range(KD):
                            fT = fT_s[:, k, :] if k < KH else fT_d[:, k - KH, :]
                            nc.tensor.matmul(
                                ps[:, half * NH : (half + 1) * NH],
                                w1_sb[:, k, m * P : (m + 1) * P],
                                fT,
                                start=(k == 0),
                                stop=(k == KD - 1),
                            )
                    for half in range(2):
                        m = 2 * pair + half
                        nc.scalar.activation(
                            hT[:, m, :],
                            ps[:, half * NH : (half + 1) * NH],
                            mybir.ActivationFunctionType.Relu,
                            bias=b1_pp[:, m : m + 1],
                        )
                return hT

            def stage_L2(s, hT):
                """Layer 2 per 128-edge group, +b2, interleaved store."""
                q = P // SUP
                for g in range(SUP):
                    ps2 = ps2_pool.tile([P, D_HID], f32, tag="ps2")
                    for half in range(2):
                        for k in range(KH):
                            nc.tensor.matmul(
                                ps2[:, half * NH : (half + 1) * NH],
                                hT[:, k, g * P : (g + 1) * P],
                                w2_sb[:, k, half * NH : (half + 1) * NH],
                                start=(k == 0),
                                stop=(k == KH - 1),
                            )
                    o_sb = out_pool.tile([P, D_HID], f32, tag="osb")
                    nc.vector.tensor_add(o_sb[:], ps2[:], b2_bc[:])
                    nc.sync.dma_start(out_r[s, g * q : (g + 1) * q], o_sb[:])

            # software pipeline: prefetch 2 supertiles ahead; L2 trails L1 by
            # one supertile so layer-2 matmuls never wait on the relu chain.
            fTs = {0: stage_P(0)}
            if S > 1:
                fTs[1] = stage_P(1)
            hTs = {0: stage_L1(0, fTs.pop(0))}
            for s in range(S):
                if s + 2 < S:
                    fTs[s + 2] = stage_P(s + 2)
                if s + 1 < S:
                    hTs[s + 1] = stage_L1(s + 1, fTs.pop(s + 1))
                stage_L2(s, hTs.pop(s))

    nc.compile()
    return nc


LAST_RESULTS = None


def kernel(**inputs):
    global LAST_RESULTS
    from concourse.bass_utils import run_bass_kernel_spmd

    node_feats = np.ascontiguousarray(np.asarray(inputs["node_feats"], np.float32))
    W1 = np.ascontiguousarray(np.asarray(inputs["W1"], np.float32))
    W2 = np.ascontiguousarray(np.asarray(inputs["W2"], np.float32))
    b1 = np.ascontiguousarray(np.asarray(inputs["b1"], np.float32))
    b2 = np.ascontiguousarray(np.asarray(inputs["b2"], np.float32))
    src = np.ascontiguousarray(np.asarray(inputs["src"]).astype(np.int32))
    dst = np.ascontiguousarray(np.asarray(inputs["dst"]).astype(np.int32))

    nc = build_nc()

    in_maps = []
    for c in range(N_CORES):
        sl = slice(c * E_CORE, (c + 1) * E_CORE)
        in_maps.append(
            {
                "node_feats": node_feats,
                "W1": W1,
                "W2": W2,
                "b1": b1,
                "b2": b2,
                "src": src[sl],
                "dst": dst[sl],
            }
        )

    trace = bool(int(os.environ.get("KERNEL_TRACE", "0")))
    kw = {}
    if trace and bool(int(os.environ.get("KERNEL_TRACE_ALL", "0"))):
        kw["trace_cores"] = list(range(N_CORES))
    res = run_bass_kernel_spmd(
        nc, in_maps, core_ids=list(range(N_CORES)), trace=trace, **kw
    )
    LAST_RESULTS = res
    return np.concatenate([r["out"] for r in res.results], axis=0)
